# revision 11
# baseline (speedup 1.0000x reference)
"""LSEP loss kernel for Trainium2, data-parallel over 8 NeuronCores.

loss_i = log(1 + (sum_{t=0} exp(x)) * (sum_{t=1} exp(-x)));  output = mean_i.

Per-core (512 rows): a = x - BIG*t, S_neg = sum exp(a),
S_pos = sum exp(-a - BIG), loss = ln(1 + S_neg*S_pos).

Structure (raw bass, 3 engines):
  SP   issues all chunk DMAs ahead of compute (per-slot DMA semaphores;
       the issuer throttles only on compute consumption, never on DMA
       completion, so the 16 SDMA engines stream back-to-back at the
       ~355 GB/s HBM floor).
  DVE  per chunk: tm = t * -BIG (2x-mode int32 tensor_scalar),
       a = x + tm; per pass: column reduces + S_neg*S_pos.
  ACT  per chunk: exp(a) and exp(-a - BIG) with accum_out col sums;
       per pass: ln(1 + prod).
Chunks are [128, 1024] (32 per pass) with 8-slot round-robin buffering:
small chunks keep the post-last-DMA serial tail short, deep buffering
keeps DMA saturated. Measured ~97 us/pass/core on HW vs ~95 us pure-DMA
floor (32 MiB/core at ~354 GB/s).
"""

from contextlib import ExitStack

import numpy as np
import concourse.bass as bass
import concourse.mybir as mybir
from concourse.bass_utils import run_bass_kernel_spmd

B, C = 4096, 8192
N_CORES = 8
ROWS = B // N_CORES   # 512 rows per core
P = 128
NPT = ROWS // P       # 4 partition tiles
FD = 1024             # free-dim chunk
NCH = C // FD         # 8 chunks per row-tile
NCHUNKS = NPT * NCH   # 32 chunks per pass, col index = p*NCH + ch
SLOTS = 8
BIG = 1024.0

F32 = mybir.dt.float32
I32 = mybir.dt.int32
AF = mybir.ActivationFunctionType
AX = mybir.AxisListType.X


TAPER = (1024,)


def _chunk_table():
    """Per-chunk (p_tile, col_start, fd). TAPER can split the last 1024
    columns into smaller final chunks to shorten the post-last-DMA serial
    chain, but measured/simulated per-chunk fixed ACT/DVE overhead (~1 us
    per extra boundary) exceeds the savings, so it stays unsplit."""
    chunks = []
    for pt in range(NPT):
        if pt < NPT - 1:
            for ch in range(NCH):
                chunks.append((pt, ch * FD, FD))
        else:
            for ch in range(NCH - 1):
                chunks.append((pt, ch * FD, FD))
            c0 = (NCH - 1) * FD
            for fd in TAPER:
                chunks.append((pt, c0, fd))
                c0 += fd
    return chunks


def build_bass(repeats=1, serialize=False):
    # repeats>1 re-runs the whole pass (DMA + compute + finalize) over the
    # same data inside one NEFF execution — used for device-time
    # measurement. serialize=True adds a cross-pass barrier (SP holds pass
    # p+1's first DMA until pass p fully finished) so the per-repeat slope
    # measures single-problem latency instead of pipelined throughput.
    chunks = _chunk_table()
    NC_ = len(chunks)
    NT = repeats * NC_
    DVE_PER_PASS = NC_ + 1       # chunk incs + prod inc
    ACT_PER_PASS = 2 * NC_ + 1   # exp incs + ln inc
    # chunk count per p_tile (last tile has the tapered extras)
    tile_last_chunk = {}
    for idx, (pt, _, _) in enumerate(chunks):
        tile_last_chunk[pt] = idx
    # DVE folds tile pt's col sums a few chunks after the tile completes so
    # the act_done wait is already satisfied (ACT lags DVE by ~1 chunk) and
    # DVE never stalls; only the last tile reduces immediately (tail).
    REDUCE_LAG = 3
    reduce_after = {}  # chunk idx -> list of p_tiles to fold
    for pt in range(NPT):
        at = tile_last_chunk[pt] if pt == NPT - 1 else min(
            tile_last_chunk[pt] + REDUCE_LAG, NC_ - 2
        )
        reduce_after.setdefault(at, []).append(pt)

    nc = bass.Bass()
    x = nc.declare_dram_parameter("inputs", [ROWS, C], F32, isOutput=False)
    t = nc.declare_dram_parameter("targets", [ROWS, C], I32, isOutput=False)
    loss = nc.declare_dram_parameter("loss", [P, NPT], F32, isOutput=True)

    with ExitStack() as ctx:
        def sb(name, shape, dt):
            return ctx.enter_context(nc.sbuf_tensor(name, shape, dt))

        xt = [sb(f"xt{i}", [P, FD], F32) for i in range(SLOTS)]
        tt = [sb(f"tt{i}", [P, FD], I32) for i in range(SLOTS)]
        aa = [sb(f"aa{i}", [P, FD], F32) for i in range(SLOTS)]
        tm = sb("tm", [P, FD], F32)       # DVE-private scratch (serial reuse)
        scr = sb("scr", [P, FD], F32)     # ACT-private exp sink (serial reuse)
        snegs = sb("snegs", [P, NC_], F32)
        sposs = sb("sposs", [P, NC_], F32)
        neg_big = sb("neg_big", [P, 1], F32)
        ssum = sb("ssum", [P, 2 * NPT], F32)
        prod = sb("prod", [P, NPT], F32)
        loss_t = sb("loss_t", [P, NPT], F32)
        dma_done = [
            ctx.enter_context(nc.semaphore(name=f"dma_done{i}"))
            for i in range(SLOTS)
        ]
        dve_done = ctx.enter_context(nc.semaphore())
        act_done = ctx.enter_context(nc.semaphore())
        out_done = ctx.enter_context(nc.semaphore())
        block = ctx.enter_context(nc.Block())

        def chunk_slice(c):
            pt, c0, fd = chunks[c]
            return slice(pt * P, (pt + 1) * P), slice(c0, c0 + fd)

        # first/last snegs column belonging to each p_tile (contiguous)
        tile_cols = {}
        for idx, (pt, _, _) in enumerate(chunks):
            lo, _ = tile_cols.get(pt, (idx, idx))
            tile_cols[pt] = (lo, idx + 1)

        # dve_done value after DVE has fully consumed global chunk j
        def dve_after_chunk(j):
            ps, c = divmod(j, NC_)
            return 1 + ps * DVE_PER_PASS + (c + 1)

        # act_done value after both exps of global chunk j
        def act_after_chunk(j):
            ps, c = divmod(j, NC_)
            return ps * ACT_PER_PASS + 2 * (c + 1)

        @block.sync
        def _(sync):
            for i in range(NT):
                s = i % SLOTS
                if serialize and i % NC_ == 0 and i > 0:
                    # pass barrier: previous pass fully done (incl. its ln)
                    sync.wait_ge(act_done, (i // NC_) * ACT_PER_PASS)
                if i >= SLOTS:
                    # xt/tt[s] free once chunk i-SLOTS's DVE consumed them
                    sync.wait_ge(dve_done, dve_after_chunk(i - SLOTS))
                rows, cols = chunk_slice(i % NC_)
                fd = chunks[i % NC_][2]
                sync.dma_start(out=xt[s][:, :fd], in_=x[rows, cols]).then_inc(
                    dma_done[s], 16
                )
                sync.dma_start(out=tt[s][:, :fd], in_=t[rows, cols]).then_inc(
                    dma_done[s], 16
                )
            sync.wait_ge(act_done, repeats * ACT_PER_PASS)
            sync.dma_start(out=loss[:, :], in_=loss_t[:, :]).then_inc(out_done, 16)
            sync.wait_ge(out_done, 16)

        @block.vector
        def _(vector):
            nc.vector.memset(neg_big[:, :], -BIG).then_inc(dve_done, 1)
            for i in range(NT):
                s = i % SLOTS
                c = i % NC_
                ps = i // NC_
                pt, _, fd = chunks[c]
                vector.wait_ge(dma_done[s], 32 * (i // SLOTS + 1))
                if i >= SLOTS:
                    # aa[s] still read by chunk i-SLOTS's exps
                    vector.wait_ge(act_done, act_after_chunk(i - SLOTS))
                nc.vector.tensor_scalar_mul(tm[:, :fd], tt[s][:, :fd], -BIG)
                nc.vector.drain()
                nc.vector.tensor_add(
                    aa[s][:, :fd], xt[s][:, :fd], tm[:, :fd]
                ).then_inc(dve_done, 1)
                for rpt in reduce_after.get(c, ()):
                    lo, hi = tile_cols[rpt]
                    vector.wait_ge(act_done, ps * ACT_PER_PASS + 2 * hi)
                    nc.vector.reduce_sum(
                        ssum[:, rpt : rpt + 1], snegs[:, lo:hi], axis=AX
                    )
                    nc.vector.reduce_sum(
                        ssum[:, NPT + rpt : NPT + rpt + 1],
                        sposs[:, lo:hi],
                        axis=AX,
                    )
                    if rpt == NPT - 1:
                        nc.vector.drain()
                        nc.vector.tensor_mul(
                            prod[:, :], ssum[:, 0:NPT], ssum[:, NPT : 2 * NPT]
                        ).then_inc(dve_done, 1)

        @block.scalar
        def _(scalar):
            for i in range(NT):
                s = i % SLOTS
                scalar.wait_ge(dve_done, dve_after_chunk(i))
                col = i % NC_
                fd = chunks[col][2]
                nc.scalar.activation(
                    scr[:, :fd], aa[s][:, :fd], AF.Exp,
                    accum_out=snegs[:, col : col + 1],
                ).then_inc(act_done, 1)
                nc.scalar.drain()
                nc.scalar.activation(
                    scr[:, :fd], aa[s][:, :fd], AF.Exp,
                    scale=-1.0, bias=neg_big[:, 0:1],
                    accum_out=sposs[:, col : col + 1],
                ).then_inc(act_done, 1)
                nc.scalar.drain()
                if col == NC_ - 1:
                    ps = i // NC_
                    # prod ready once DVE finished this pass's final TT
                    scalar.wait_ge(dve_done, 1 + (ps + 1) * DVE_PER_PASS)
                    nc.scalar.activation(
                        loss_t[:, :], prod[:, :], AF.Ln, bias=1.0
                    ).then_inc(act_done, 1)
                    nc.scalar.drain()

    return nc


_NC_CACHE = []


def _get_nc():
    if not _NC_CACHE:
        _NC_CACHE.append(build_bass())
    return _NC_CACHE[0]


def _run(inputs, targets, trace=False, **kw):
    nc = _get_nc()
    in_maps = [
        {
            "inputs": np.ascontiguousarray(inputs[i * ROWS : (i + 1) * ROWS]),
            "targets": np.ascontiguousarray(targets[i * ROWS : (i + 1) * ROWS]),
        }
        for i in range(N_CORES)
    ]
    res = run_bass_kernel_spmd(nc, in_maps, list(range(N_CORES)), trace=trace, **kw)
    # loss tensor is [partition, p_tile]; row r of this core's shard = p_tile*128 + partition
    losses = np.concatenate(
        [res.results[i]["loss"].T.reshape(-1) for i in range(N_CORES)]
    )
    out = np.float32(np.mean(losses.astype(np.float64)))
    return out, res


def kernel(inputs: np.ndarray, targets: np.ndarray) -> np.ndarray:
    out, _ = _run(np.asarray(inputs), np.asarray(targets))
    return out


# revision 14
# speedup vs baseline: 1.1651x; 1.1651x over previous
"""LSEP loss kernel for Trainium2, data-parallel over 8 NeuronCores.

loss_i = log(1 + (sum_{t=0} exp(x)) * (sum_{t=1} exp(-x)));  output = mean_i.

Per-core (512 rows): a = x - BIG*t, S_neg = sum exp(a),
S_pos = sum exp(-a - BIG), loss = ln(1 + S_neg*S_pos).

Structure (raw bass, 3 engines):
  SP   issues all chunk DMAs ahead of compute (per-slot DMA semaphores;
       the issuer throttles only on compute consumption, never on DMA
       completion, so the 16 SDMA engines stream back-to-back at the
       ~355 GB/s HBM floor).
  DVE  per chunk: tm = t * -BIG (2x-mode int32 tensor_scalar),
       a = x + tm; per pass: column reduces + S_neg*S_pos.
  ACT  per chunk: exp(a) and exp(-a - BIG) with accum_out col sums;
       per pass: ln(1 + prod).
Chunks are [128, 1024] (32 per pass) with 16-slot round-robin
buffering: small chunks keep the post-last-DMA serial tail short, deep
buffering keeps DMA saturated. Explicit per-op drains are off — the
engines interlock output hazards themselves (verified bit-identical) and
the extra sequencer instructions measurably cost time. Measured ~98
us/pass/core pipelined (~106 us serialized single-problem latency) on HW
vs ~96 us pure-DMA floor (32 MiB/core at ~355 GB/s).
"""

from contextlib import ExitStack

import numpy as np
import concourse.bass as bass
import concourse.mybir as mybir
from concourse.bass_utils import run_bass_kernel_spmd

B, C = 4096, 8192
N_CORES = 8
ROWS = B // N_CORES   # 512 rows per core
P = 128
NPT = ROWS // P       # 4 partition tiles
FD = 1024             # free-dim chunk
NCH = C // FD         # 8 chunks per row-tile
NCHUNKS = NPT * NCH   # 32 chunks per pass, col index = p*NCH + ch
SLOTS = 16
BIG = 1024.0

F32 = mybir.dt.float32
I32 = mybir.dt.int32
AF = mybir.ActivationFunctionType
AX = mybir.AxisListType.X


TAPER = (1024,)


def _chunk_table():
    """Per-chunk (p_tile, col_start, fd). TAPER can split the last 1024
    columns into smaller final chunks to shorten the post-last-DMA serial
    chain, but measured/simulated per-chunk fixed ACT/DVE overhead (~1 us
    per extra boundary) exceeds the savings, so it stays unsplit."""
    chunks = []
    for pt in range(NPT):
        if pt < NPT - 1:
            for ch in range(NCH):
                chunks.append((pt, ch * FD, FD))
        else:
            for ch in range(NCH - 1):
                chunks.append((pt, ch * FD, FD))
            c0 = (NCH - 1) * FD
            for fd in TAPER:
                chunks.append((pt, c0, fd))
                c0 += fd
    return chunks


def build_bass(repeats=1, serialize=False, slots=SLOTS, drains=False):
    # repeats>1 re-runs the whole pass (DMA + compute + finalize) over the
    # same data inside one NEFF execution — used for device-time
    # measurement. serialize=True adds a cross-pass barrier (SP holds pass
    # p+1's first DMA until pass p fully finished) so the per-repeat slope
    # measures single-problem latency instead of pipelined throughput.
    chunks = _chunk_table()
    SLOTS = slots
    NC_ = len(chunks)
    NT = repeats * NC_
    DVE_PER_PASS = NC_ + 1       # chunk incs + prod inc
    ACT_PER_PASS = 2 * NC_ + 1   # exp incs + ln inc
    # chunk count per p_tile (last tile has the tapered extras)
    tile_last_chunk = {}
    for idx, (pt, _, _) in enumerate(chunks):
        tile_last_chunk[pt] = idx
    # DVE folds tile pt's col sums REDUCE_LAG chunks after the tile
    # completes. Small lags stall DVE on HW (ACT runs closer to the DMA
    # rate than the sim models, so the act_done wait bites and the stall
    # backs up into the DMA pipeline — measured +14 us/pass at lag 3).
    # A full-pass lag clamps every fold to the end of the pass, where the
    # single unavoidable wait already exists.
    REDUCE_LAG = NC_
    reduce_after = {}  # chunk idx -> list of p_tiles to fold
    for pt in range(NPT):
        at = tile_last_chunk[pt] if pt == NPT - 1 else min(
            tile_last_chunk[pt] + REDUCE_LAG, NC_ - 2
        )
        reduce_after.setdefault(at, []).append(pt)

    nc = bass.Bass()
    x = nc.declare_dram_parameter("inputs", [ROWS, C], F32, isOutput=False)
    t = nc.declare_dram_parameter("targets", [ROWS, C], I32, isOutput=False)
    loss = nc.declare_dram_parameter("loss", [P, NPT], F32, isOutput=True)

    with ExitStack() as ctx:
        def sb(name, shape, dt):
            return ctx.enter_context(nc.sbuf_tensor(name, shape, dt))

        xt = [sb(f"xt{i}", [P, FD], F32) for i in range(SLOTS)]
        tt = [sb(f"tt{i}", [P, FD], I32) for i in range(SLOTS)]
        aa = [sb(f"aa{i}", [P, FD], F32) for i in range(SLOTS)]
        tm = sb("tm", [P, FD], F32)       # DVE-private scratch (serial reuse)
        scr = sb("scr", [P, FD], F32)     # ACT-private exp sink (serial reuse)
        snegs = sb("snegs", [P, NC_], F32)
        sposs = sb("sposs", [P, NC_], F32)
        neg_big = sb("neg_big", [P, 1], F32)
        ssum = sb("ssum", [P, 2 * NPT], F32)
        prod = sb("prod", [P, NPT], F32)
        loss_t = sb("loss_t", [P, NPT], F32)
        dma_done = [
            ctx.enter_context(nc.semaphore(name=f"dma_done{i}"))
            for i in range(SLOTS)
        ]
        dve_done = ctx.enter_context(nc.semaphore())
        act_done = ctx.enter_context(nc.semaphore())
        out_done = ctx.enter_context(nc.semaphore())
        block = ctx.enter_context(nc.Block())

        def chunk_slice(c):
            pt, c0, fd = chunks[c]
            return slice(pt * P, (pt + 1) * P), slice(c0, c0 + fd)

        # first/last snegs column belonging to each p_tile (contiguous)
        tile_cols = {}
        for idx, (pt, _, _) in enumerate(chunks):
            lo, _ = tile_cols.get(pt, (idx, idx))
            tile_cols[pt] = (lo, idx + 1)

        # dve_done value after DVE has fully consumed global chunk j
        def dve_after_chunk(j):
            ps, c = divmod(j, NC_)
            return 1 + ps * DVE_PER_PASS + (c + 1)

        # act_done value after both exps of global chunk j
        def act_after_chunk(j):
            ps, c = divmod(j, NC_)
            return ps * ACT_PER_PASS + 2 * (c + 1)

        @block.sync
        def _(sync):
            for i in range(NT):
                s = i % SLOTS
                if serialize and i % NC_ == 0 and i > 0:
                    # pass barrier: previous pass fully done (incl. its ln)
                    sync.wait_ge(act_done, (i // NC_) * ACT_PER_PASS)
                if i >= SLOTS:
                    # xt/tt[s] free once chunk i-SLOTS's DVE consumed them
                    sync.wait_ge(dve_done, dve_after_chunk(i - SLOTS))
                rows, cols = chunk_slice(i % NC_)
                fd = chunks[i % NC_][2]
                sync.dma_start(out=xt[s][:, :fd], in_=x[rows, cols]).then_inc(
                    dma_done[s], 16
                )
                sync.dma_start(out=tt[s][:, :fd], in_=t[rows, cols]).then_inc(
                    dma_done[s], 16
                )
            sync.wait_ge(act_done, repeats * ACT_PER_PASS)
            sync.dma_start(out=loss[:, :], in_=loss_t[:, :]).then_inc(out_done, 16)
            sync.wait_ge(out_done, 16)

        @block.vector
        def _(vector):
            nc.vector.memset(neg_big[:, :], -BIG).then_inc(dve_done, 1)
            for i in range(NT):
                s = i % SLOTS
                c = i % NC_
                ps = i // NC_
                pt, _, fd = chunks[c]
                vector.wait_ge(dma_done[s], 32 * (i // SLOTS + 1))
                if i >= SLOTS:
                    # aa[s] still read by chunk i-SLOTS's exps
                    vector.wait_ge(act_done, act_after_chunk(i - SLOTS))
                nc.vector.tensor_scalar_mul(tm[:, :fd], tt[s][:, :fd], -BIG)
                if drains:
                    nc.vector.drain()
                nc.vector.tensor_add(
                    aa[s][:, :fd], xt[s][:, :fd], tm[:, :fd]
                ).then_inc(dve_done, 1)
                for rpt in reduce_after.get(c, ()):
                    lo, hi = tile_cols[rpt]
                    vector.wait_ge(act_done, ps * ACT_PER_PASS + 2 * hi)
                    nc.vector.reduce_sum(
                        ssum[:, rpt : rpt + 1], snegs[:, lo:hi], axis=AX
                    )
                    nc.vector.reduce_sum(
                        ssum[:, NPT + rpt : NPT + rpt + 1],
                        sposs[:, lo:hi],
                        axis=AX,
                    )
                    if rpt == NPT - 1:
                        nc.vector.drain()
                        nc.vector.tensor_mul(
                            prod[:, :], ssum[:, 0:NPT], ssum[:, NPT : 2 * NPT]
                        ).then_inc(dve_done, 1)

        @block.scalar
        def _(scalar):
            for i in range(NT):
                s = i % SLOTS
                scalar.wait_ge(dve_done, dve_after_chunk(i))
                col = i % NC_
                fd = chunks[col][2]
                nc.scalar.activation(
                    scr[:, :fd], aa[s][:, :fd], AF.Exp,
                    accum_out=snegs[:, col : col + 1],
                ).then_inc(act_done, 1)
                if drains:
                    nc.scalar.drain()
                nc.scalar.activation(
                    scr[:, :fd], aa[s][:, :fd], AF.Exp,
                    scale=-1.0, bias=neg_big[:, 0:1],
                    accum_out=sposs[:, col : col + 1],
                ).then_inc(act_done, 1)
                if drains:
                    nc.scalar.drain()
                if col == NC_ - 1:
                    ps = i // NC_
                    # prod ready once DVE finished this pass's final TT
                    scalar.wait_ge(dve_done, 1 + (ps + 1) * DVE_PER_PASS)
                    nc.scalar.activation(
                        loss_t[:, :], prod[:, :], AF.Ln, bias=1.0
                    ).then_inc(act_done, 1)
                    nc.scalar.drain()

    return nc


_NC_CACHE = []


def _get_nc():
    if not _NC_CACHE:
        _NC_CACHE.append(build_bass())
    return _NC_CACHE[0]


def _run(inputs, targets, trace=False, **kw):
    nc = _get_nc()
    in_maps = [
        {
            "inputs": np.ascontiguousarray(inputs[i * ROWS : (i + 1) * ROWS]),
            "targets": np.ascontiguousarray(targets[i * ROWS : (i + 1) * ROWS]),
        }
        for i in range(N_CORES)
    ]
    res = run_bass_kernel_spmd(nc, in_maps, list(range(N_CORES)), trace=trace, **kw)
    # loss tensor is [partition, p_tile]; row r of this core's shard = p_tile*128 + partition
    losses = np.concatenate(
        [res.results[i]["loss"].T.reshape(-1) for i in range(N_CORES)]
    )
    out = np.float32(np.mean(losses.astype(np.float64)))
    return out, res


def kernel(inputs: np.ndarray, targets: np.ndarray) -> np.ndarray:
    out, _ = _run(np.asarray(inputs), np.asarray(targets))
    return out


# revision 17
# speedup vs baseline: 1.1761x; 1.0094x over previous
"""LSEP loss kernel for Trainium2, data-parallel over 8 NeuronCores.

loss_i = log(1 + (sum_{t=0} exp(x)) * (sum_{t=1} exp(-x)));  output = mean_i.

Per-core (512 rows): a = x - BIG*t, S_neg = sum exp(a),
S_pos = sum exp(-a - BIG), loss = ln(1 + S_neg*S_pos).

Structure (raw bass, 3 engines):
  SP   issues all chunk DMAs ahead of compute (per-slot DMA semaphores;
       the issuer throttles only on compute consumption, never on DMA
       completion, so the 16 SDMA engines stream back-to-back at the
       ~355 GB/s HBM floor).
  DVE  per chunk: tm = t * -BIG (2x-mode int32 tensor_scalar),
       a = x + tm; per pass: column reduces + S_neg*S_pos.
  ACT  per chunk: exp(a) and exp(-a - BIG) with accum_out col sums;
       per pass: ln(1 + prod).
Chunks are [128, 1024] (32 per pass) with 16-slot round-robin
buffering: small chunks keep the post-last-DMA serial tail short, deep
buffering keeps DMA saturated. Explicit per-op drains are off — the
engines interlock output hazards themselves (verified bit-identical) and
the extra sequencer instructions measurably cost time. Measured ~98
us/pass/core pipelined (~106 us serialized single-problem latency) on HW
vs ~96 us pure-DMA floor (32 MiB/core at ~355 GB/s).
"""

from contextlib import ExitStack

import numpy as np
import concourse.bass as bass
import concourse.mybir as mybir
from concourse.bass_utils import run_bass_kernel_spmd

B, C = 4096, 8192
N_CORES = 8
ROWS = B // N_CORES   # 512 rows per core
P = 128
NPT = ROWS // P       # 4 partition tiles
FD = 1024             # free-dim chunk
NCH = C // FD         # 8 chunks per row-tile
NCHUNKS = NPT * NCH   # 32 chunks per pass, col index = p*NCH + ch
SLOTS = 16
BIG = 1024.0

F32 = mybir.dt.float32
I32 = mybir.dt.int32
AF = mybir.ActivationFunctionType
AX = mybir.AxisListType.X


TAPER = (1024,)


def _chunk_table():
    """Per-chunk (p_tile, col_start, fd). TAPER can split the last 1024
    columns into smaller final chunks to shorten the post-last-DMA serial
    chain, but measured/simulated per-chunk fixed ACT/DVE overhead (~1 us
    per extra boundary) exceeds the savings, so it stays unsplit."""
    chunks = []
    for pt in range(NPT):
        if pt < NPT - 1:
            for ch in range(NCH):
                chunks.append((pt, ch * FD, FD))
        else:
            for ch in range(NCH - 1):
                chunks.append((pt, ch * FD, FD))
            c0 = (NCH - 1) * FD
            for fd in TAPER:
                chunks.append((pt, c0, fd))
                c0 += fd
    return chunks


def build_bass(repeats=1, serialize=False, slots=SLOTS, drains=False, fused=False):
    # repeats>1 re-runs the whole pass (DMA + compute + finalize) over the
    # same data inside one NEFF execution — used for device-time
    # measurement. serialize=True adds a cross-pass barrier (SP holds pass
    # p+1's first DMA until pass p fully finished) so the per-repeat slope
    # measures single-problem latency instead of pipelined throughput.
    chunks = _chunk_table()
    SLOTS = slots
    NC_ = len(chunks)
    NT = repeats * NC_
    DVE_PER_PASS = NC_ + 1       # chunk incs + prod inc
    ACT_PER_PASS = 2 * NC_ + 1   # exp incs + ln inc
    # chunk count per p_tile (last tile has the tapered extras)
    tile_last_chunk = {}
    for idx, (pt, _, _) in enumerate(chunks):
        tile_last_chunk[pt] = idx
    # DVE folds tile pt's col sums REDUCE_LAG chunks after the tile
    # completes. Small lags stall DVE on HW (ACT runs closer to the DMA
    # rate than the sim models, so the act_done wait bites and the stall
    # backs up into the DMA pipeline — measured +14 us/pass at lag 3).
    # A full-pass lag clamps every fold to the end of the pass, where the
    # single unavoidable wait already exists.
    REDUCE_LAG = NC_
    reduce_after = {}  # chunk idx -> list of p_tiles to fold
    for pt in range(NPT):
        at = tile_last_chunk[pt] if pt == NPT - 1 else min(
            tile_last_chunk[pt] + REDUCE_LAG, NC_ - 2
        )
        reduce_after.setdefault(at, []).append(pt)

    nc = bass.Bass()
    x = nc.declare_dram_parameter("inputs", [ROWS, C], F32, isOutput=False)
    t = nc.declare_dram_parameter("targets", [ROWS, C], I32, isOutput=False)
    loss = nc.declare_dram_parameter("loss", [P, NPT], F32, isOutput=True)

    with ExitStack() as ctx:
        def sb(name, shape, dt):
            return ctx.enter_context(nc.sbuf_tensor(name, shape, dt))

        xt = [sb(f"xt{i}", [P, FD], F32) for i in range(SLOTS)]
        tt = [sb(f"tt{i}", [P, FD], I32) for i in range(SLOTS)]
        aa = [sb(f"aa{i}", [P, FD], F32) for i in range(SLOTS)]
        tm = sb("tm", [P, FD], F32)       # DVE-private scratch (serial reuse)
        scr = sb("scr", [P, FD], F32)     # ACT-private exp sink (serial reuse)
        snegs = sb("snegs", [P, NC_], F32)
        sposs = sb("sposs", [P, NC_], F32)
        neg_big = sb("neg_big", [P, 1], F32)
        ssum = sb("ssum", [P, 2 * NPT], F32)
        prod = sb("prod", [P, NPT], F32)
        loss_t = sb("loss_t", [P, NPT], F32)
        dma_x = [
            ctx.enter_context(nc.semaphore(name=f"dma_x{i}"))
            for i in range(SLOTS)
        ]
        dma_t = [
            ctx.enter_context(nc.semaphore(name=f"dma_t{i}"))
            for i in range(SLOTS)
        ]
        dve_done = ctx.enter_context(nc.semaphore())
        act_done = ctx.enter_context(nc.semaphore())
        out_done = ctx.enter_context(nc.semaphore())
        block = ctx.enter_context(nc.Block())

        def chunk_slice(c):
            pt, c0, fd = chunks[c]
            return slice(pt * P, (pt + 1) * P), slice(c0, c0 + fd)

        # first/last snegs column belonging to each p_tile (contiguous)
        tile_cols = {}
        for idx, (pt, _, _) in enumerate(chunks):
            lo, _ = tile_cols.get(pt, (idx, idx))
            tile_cols[pt] = (lo, idx + 1)

        # dve_done value after DVE has fully consumed global chunk j
        def dve_after_chunk(j):
            ps, c = divmod(j, NC_)
            return 1 + ps * DVE_PER_PASS + (c + 1)

        # act_done value after both exps of global chunk j
        def act_after_chunk(j):
            ps, c = divmod(j, NC_)
            return ps * ACT_PER_PASS + 2 * (c + 1)

        @block.sync
        def _(sync):
            for i in range(NT):
                s = i % SLOTS
                if serialize and i % NC_ == 0 and i > 0:
                    # pass barrier: previous pass fully done (incl. its ln)
                    sync.wait_ge(act_done, (i // NC_) * ACT_PER_PASS)
                if i >= SLOTS:
                    # xt/tt[s] free once chunk i-SLOTS's DVE consumed them
                    sync.wait_ge(dve_done, dve_after_chunk(i - SLOTS))
                rows, cols = chunk_slice(i % NC_)
                fd = chunks[i % NC_][2]
                # t first: DVE's mask op needs only t, so it can start while
                # x is still in flight (takes the mask off the tail's
                # critical path)
                sync.dma_start(out=tt[s][:, :fd], in_=t[rows, cols]).then_inc(
                    dma_t[s], 16
                )
                sync.dma_start(out=xt[s][:, :fd], in_=x[rows, cols]).then_inc(
                    dma_x[s], 16
                )
            sync.wait_ge(act_done, repeats * ACT_PER_PASS)
            sync.dma_start(out=loss[:, :], in_=loss_t[:, :]).then_inc(out_done, 16)
            sync.wait_ge(out_done, 16)

        @block.vector
        def _(vector):
            nc.vector.memset(neg_big[:, :], -BIG).then_inc(dve_done, 1)
            for i in range(NT):
                s = i % SLOTS
                c = i % NC_
                ps = i // NC_
                pt, _, fd = chunks[c]
                vector.wait_ge(dma_t[s], 16 * (i // SLOTS + 1))
                if i >= SLOTS:
                    # aa[s] still read by chunk i-SLOTS's exps
                    vector.wait_ge(act_done, act_after_chunk(i - SLOTS))
                if fused:
                    # one-pass custom DVE op: aa = (t * -BIG + 0) + x
                    vector.wait_ge(dma_x[s], 16 * (i // SLOTS + 1))
                    nc.vector.affine_then_add(
                        aa[s][:, :fd], tt[s][:, :fd], xt[s][:, :fd], -BIG, 0.0
                    ).then_inc(dve_done, 1)
                else:
                    nc.vector.tensor_scalar_mul(tm[:, :fd], tt[s][:, :fd], -BIG)
                    if drains:
                        nc.vector.drain()
                    vector.wait_ge(dma_x[s], 16 * (i // SLOTS + 1))
                    nc.vector.tensor_add(
                        aa[s][:, :fd], xt[s][:, :fd], tm[:, :fd]
                    ).then_inc(dve_done, 1)
                for rpt in reduce_after.get(c, ()):
                    lo, hi = tile_cols[rpt]
                    vector.wait_ge(act_done, ps * ACT_PER_PASS + 2 * hi)
                    nc.vector.reduce_sum(
                        ssum[:, rpt : rpt + 1], snegs[:, lo:hi], axis=AX
                    )
                    nc.vector.reduce_sum(
                        ssum[:, NPT + rpt : NPT + rpt + 1],
                        sposs[:, lo:hi],
                        axis=AX,
                    )
                    if rpt == NPT - 1:
                        nc.vector.drain()
                        nc.vector.tensor_mul(
                            prod[:, :], ssum[:, 0:NPT], ssum[:, NPT : 2 * NPT]
                        ).then_inc(dve_done, 1)

        @block.scalar
        def _(scalar):
            for i in range(NT):
                s = i % SLOTS
                scalar.wait_ge(dve_done, dve_after_chunk(i))
                col = i % NC_
                fd = chunks[col][2]
                nc.scalar.activation(
                    scr[:, :fd], aa[s][:, :fd], AF.Exp,
                    accum_out=snegs[:, col : col + 1],
                ).then_inc(act_done, 1)
                if drains:
                    nc.scalar.drain()
                nc.scalar.activation(
                    scr[:, :fd], aa[s][:, :fd], AF.Exp,
                    scale=-1.0, bias=neg_big[:, 0:1],
                    accum_out=sposs[:, col : col + 1],
                ).then_inc(act_done, 1)
                if drains:
                    nc.scalar.drain()
                if col == NC_ - 1:
                    ps = i // NC_
                    # prod ready once DVE finished this pass's final TT
                    scalar.wait_ge(dve_done, 1 + (ps + 1) * DVE_PER_PASS)
                    nc.scalar.activation(
                        loss_t[:, :], prod[:, :], AF.Ln, bias=1.0
                    ).then_inc(act_done, 1)
                    nc.scalar.drain()

    return nc


_NC_CACHE = []


def _get_nc():
    if not _NC_CACHE:
        _NC_CACHE.append(build_bass())
    return _NC_CACHE[0]


def _run(inputs, targets, trace=False, **kw):
    nc = _get_nc()
    in_maps = [
        {
            "inputs": np.ascontiguousarray(inputs[i * ROWS : (i + 1) * ROWS]),
            "targets": np.ascontiguousarray(targets[i * ROWS : (i + 1) * ROWS]),
        }
        for i in range(N_CORES)
    ]
    res = run_bass_kernel_spmd(nc, in_maps, list(range(N_CORES)), trace=trace, **kw)
    # loss tensor is [partition, p_tile]; row r of this core's shard = p_tile*128 + partition
    losses = np.concatenate(
        [res.results[i]["loss"].T.reshape(-1) for i in range(N_CORES)]
    )
    out = np.float32(np.mean(losses.astype(np.float64)))
    return out, res


def kernel(inputs: np.ndarray, targets: np.ndarray) -> np.ndarray:
    out, _ = _run(np.asarray(inputs), np.asarray(targets))
    return out
